# revision 47
# baseline (speedup 1.0000x reference)
"""Trainium2 Bass kernel for nn_MultiHeadAttention_68959994904763.

Sharding (8 NeuronCores): 2-D tensor-parallel - batch (2) x head-groups (4).
Core c handles batch b = c // 4 and heads [4g, 4g+4) with g = c % 4.
Each core computes a partial output o_heads @ W_o for its 4 heads; the
host sums the 4 partials per batch (bf16 partials, f32 accumulate) and
adds the host-folded bias b_o_eff = b_v.flatten() @ W_o + b_o.

Per-core kernel (QKV inputs, es/va/W_o/masks/output in bf16; q/k score
path kept f32r; PSUM accumulation always fp32):
  1. Inputs DMA'd with 2KB-contiguous partition rows (one DMA per
     (kind, pair) weight tensor, window-aligned xT chunks): the input
     path is DMA-descriptor-rate-bound, not byte-bound, so descriptor
     count is what matters.  Issue order tracks first consumption.
  2. q/k projections for each pair interleaved per e-chunk over two
     4-window PSUM quads (8 banks), so each arriving xT tile feeds 8
     matmuls and the PE outpaces HBM arrival.  Bias-add alternates
     ACT identity / DVE tensor_scalar_add.
  3. v projections sw-major (both pairs per window) + PE re-transpose
     into per-(pair, s-chunk) va tiles [128, 130] = [h0|1|h1|1] (ones
     columns = softmax denominator trick), written by ONE strided copy
     per transpose, alternating ACT/DVE.  qw0's eight score groups are
     precomputed here as PE filler (sg_early ring) so the PE stream is
     dense across the QKV->attention seam and the HAM clock gate stays
     at K=8/8 (2.4 GHz) into the attention phase.
  4. Attention per (q-window, pair): both heads' score matmuls run
     CONCURRENTLY as PE row-tiles (K=64, tile_position (0,0)/(64,0))
     into the two banks of a [128, 2*NQ] PSUM group; one batched exp
     (ACT) covers both heads; 0/1 mask multiply (DVE, bf16) on
     diagonal groups only.  o-matmuls (lhsT = va 65-col slice, ones
     column -> row 64 = denominator) lag LAG=4 groups; each pair's
     last o-mms + epilogues carry into the next pair's first groups.
     W_o chunks of the previous window drip one per kc iteration.
     Epilogue: DVE row copy -> PE outer-product broadcast ->
     reciprocal_approx_fast -> DVE multiply into oT (no iterative
     reciprocal; a GpSimd partition_broadcast variant measured ~2 us
     slower overall).  PSUM: sg 2x2 + po-ring 3 (pb rides it) +
     wo 1 = 8 banks.
  5. Final window's W_o runs 4-deep pipelined on the banks freed by
     the score pool, copies alternating ACT/DVE, bf16 output.

Measured on trn2 (8 cores, NTFF): ~177-182 us typical (occasional
~+30 us outlier runs), rel err 3.8e-3 (gate 2e-2).  History: 332 us
baseline -> 244 (paired scores + fast recip) -> 213 (descriptor
coalescing) -> 203 (bf16 attention pack) -> 196 (qw0 precompute) ->
183 (va strided copies) -> 177-182 (q/k interleave).  PE is
HAM-warm for nearly the whole span; attention is PE-bound with ACT
(exp) within ~10% - wider exp batches need PSUM banks that aren't
there.  Known residue: ~10 us DMA-runway startup, ~10 us tail
(epilogue chain + final W_o + output drain), occasional throttled
outlier runs.
"""

import os
import sys
import types

import numpy as np

S, E, D = 2048, 1024, 64
P = 128
NQ = 512  # q-window (moving operand) size
SC = S // P  # 16 s-chunks
EC = E // P  # 8 e-chunks
QW = S // NQ  # 4 q-windows
N_CORES = 8


def _ensure_axon_hooks():
    """Provide antenv.axon_hooks (NTFF profile hook registry) if the image
    lacks it, and register the ctypes-based hook so trace=True works."""
    try:
        from antenv.axon_hooks import get_axon_ntff_profile_hook  # noqa: F401
        return
    except ImportError:
        pass
    import antenv

    mod = types.ModuleType("antenv.axon_hooks")
    _h = [None]
    mod.set_axon_ntff_profile_hook = lambda h: _h.__setitem__(0, h)
    mod.get_axon_ntff_profile_hook = lambda: _h[0]
    sys.modules["antenv.axon_hooks"] = mod
    antenv.axon_hooks = mod
    try:
        from trn_agent_boot.trn_boot import _ntff_profile_via_ctypes

        so_path = "/opt/axon/libaxon_pjrt.so"
        if os.path.exists(so_path):
            mod.set_axon_ntff_profile_hook(_ntff_profile_via_ctypes(so_path))
    except Exception:
        pass


def _build_program():
    import concourse.bass as bass  # noqa: F401
    import concourse.mybir as mybir
    import concourse.tile as tile
    from concourse import bacc
    import contextlib

    f32 = mybir.dt.float32
    f32r = mybir.dt.float32r
    bf16 = mybir.dt.bfloat16

    nc = bacc.Bacc("TRN2", target_bir_lowering=False, debug=False)

    # weights per (kind, pair) combined into one [P, EC*P] tensor (partition
    # row = 2KB contiguous in DRAM): input DMA is descriptor-rate-bound, so
    # fewer/bigger descriptors load ~2x faster than per-ec 256B rows
    xT_d = nc.dram_tensor("xT", [E, S], bf16, kind="ExternalInput").ap()
    wq_d = nc.dram_tensor("wq", [2, P, EC * P], bf16, kind="ExternalInput").ap()
    wk_d = nc.dram_tensor("wk", [2, P, EC * P], bf16, kind="ExternalInput").ap()
    wv_d = nc.dram_tensor("wv", [2, P, EC * P], bf16, kind="ExternalInput").ap()
    bq_d = nc.dram_tensor("bq", [2, P, 1], f32, kind="ExternalInput").ap()
    bk_d = nc.dram_tensor("bk", [2, P, 1], f32, kind="ExternalInput").ap()
    wo_d = nc.dram_tensor("wo", [2, P, E], bf16, kind="ExternalInput").ap()
    mk_d = nc.dram_tensor("masks", [P, 4 * NQ], bf16, kind="ExternalInput").ap()
    id_d = nc.dram_tensor("ident", [P, P], bf16, kind="ExternalInput").ap()
    out_d = nc.dram_tensor("out", [S, E], bf16, kind="ExternalOutput").ap()

    Act = mybir.ActivationFunctionType

    with tile.TileContext(nc) as tc:
        with contextlib.ExitStack() as top:
            persist = top.enter_context(tc.tile_pool(name="persist", bufs=1))

            # --- persistent constants / weights ---
            # (attention-phase tensors — masks, wo — are DMA'd after the
            # QKV weights so x/weight loads aren't delayed at startup)
            ident = persist.tile([P, P], bf16, tag="ident")
            bq_t, bk_t = [], []
            for pr in range(2):
                bq_t.append(persist.tile([P, 1], f32, tag=f"bq{pr}", name=f"bq{pr}"))
                bk_t.append(persist.tile([P, 1], f32, tag=f"bk{pr}", name=f"bk{pr}"))

            # persistent activations
            qT = [persist.tile([P, S], f32r, tag=f"qT{pr}", name=f"qT{pr}") for pr in range(2)]
            kT = [persist.tile([P, S], f32r, tag=f"kT{pr}", name=f"kT{pr}") for pr in range(2)]
            oT = [persist.tile([P, S], bf16, tag=f"oT{pr}", name=f"oT{pr}") for pr in range(2)]
            # v_aug per (pair, s-chunk): [128, 130] = [h0 64 | ones | h1 64
            # | ones]; each head's lhsT is the contiguous 65-col slice ending
            # in its ones column (softmax denominator trick)
            va2 = [
                [persist.tile([P, 2 * (D + 1)], bf16, tag=f"va{pr}_{sc}", name=f"va{pr}_{sc}") for sc in range(SC)]
                for pr in range(2)
            ]

            # es tiles outlive phase 1 (qw0 scores precomputed there)
            epool = top.enter_context(tc.tile_pool(name="epool", bufs=18))
            pre_es = {}

            # ---------- Phases 1+2: x^T, QKV ----------
            with contextlib.ExitStack() as ph12:

                xTp = ph12.enter_context(tc.tile_pool(name="xT", bufs=1))
                xT = [xTp.tile([P, S], bf16, tag=f"xT{ec}", name=f"xT{ec}") for ec in range(EC)]

                # DMA issue order matters: later DMAs queue behind earlier
                # ones, so emit in consumption order — pair-0 QKV weights
                # first (the very first matmul needs wq[0][0]), then x^T,
                # then pair-1 weights, then attention-phase constants
                # (wo, masks).  One dma_start per tile: multi-writer chunked
                # tiles race on HW when chunks arrive just-in-time, and
                # large DMAs already fan out across queues internally.
                wpool = ph12.enter_context(tc.tile_pool(name="wqkv", bufs=1))
                wq_t, wk_t, wv_t = [None, None], [None, None], [None, None]

                def load_w(nm, store, dram, pr):
                    # one DMA per (kind, pair): [128, 1024] bf16, 2KB rows
                    t = wpool.tile([P, EC * P], bf16, tag=f"w{nm}{pr}", name=f"w{nm}{pr}")
                    nc.sync.dma_start(t[:], dram[pr])
                    store[pr] = t

                def load_xT(ec, nchunks=1):
                    # chunks aligned to NQ-column window multiples: one
                    # writer per chunk, no reader spans a chunk boundary
                    r = slice(ec * P, (ec + 1) * P)
                    cw = S // nchunks
                    for c in range(nchunks):
                        cs = slice(c * cw, (c + 1) * cw)
                        nc.sync.dma_start(xT[ec][:, cs], xT_d[r, cs])

                # DMA issue order tracks first-consumption order: the first
                # pass only needs wq + xT, so wk/wv issue after the early xT
                # tiles.
                load_w("q", wq_t, wq_d, 0)
                load_w("k", wk_t, wk_d, 0)
                load_xT(0, nchunks=2)
                load_xT(1, nchunks=2)
                load_xT(2)
                load_xT(3)
                for pr in range(2):
                    nc.sync.dma_start(bq_t[pr][:], bq_d[pr])
                    nc.sync.dma_start(bk_t[pr][:], bk_d[pr])
                nc.sync.dma_start(ident[:], id_d[:])
                load_w("q", wq_t, wq_d, 1)
                for ec in range(4, EC):
                    load_xT(ec)
                load_w("k", wk_t, wk_d, 1)
                m_all = persist.tile([P, 4 * NQ], bf16, tag="masks", name="m_all")
                nc.sync.dma_start(m_all[:], mk_d[:])
                mask_t = [m_all[:, j * NQ : (j + 1) * NQ] for j in range(4)]
                load_w("v", wv_t, wv_d, 0)
                load_w("v", wv_t, wv_d, 1)
                wo_t = []
                for pr in range(2):
                    t = persist.tile([P, E], bf16, tag=f"wo{pr}", name=f"wo{pr}")
                    nc.sync.dma_start(t[:], wo_d[pr])
                    wo_t.append(t)

                vtmp = ph12.enter_context(tc.tile_pool(name="vtmp", bufs=2))

                # --- q/k passes: q and k interleaved per e-chunk (two
                # 4-window quads, 8 banks) so each arriving xT tile feeds 8
                # matmuls: the PE outpaces HBM arrival instead of stalling ---
                with contextlib.ExitStack() as ph_qk:
                    ps_qk = ph_qk.enter_context(
                        tc.tile_pool(name="ps_qk", bufs=1, space="PSUM")
                    )
                    for pr in range(2):
                        pq2 = {
                            kind: [
                                ps_qk.tile([P, NQ], f32, tag=f"pq{kind}{sw}", name=f"pq{kind}{sw}")
                                for sw in range(QW)
                            ]
                            for kind in ("q", "k")
                        }
                        for ec in range(EC):
                            for kind, w_t in (("q", wq_t[pr]), ("k", wk_t[pr])):
                                for sw in range(QW):
                                    nc.tensor.matmul(
                                        pq2[kind][sw][:],
                                        w_t[:, ec * P : (ec + 1) * P],
                                        xT[ec][:, sw * NQ : (sw + 1) * NQ],
                                        start=(ec == 0),
                                        stop=(ec == EC - 1),
                                    )
                        for kind, dst, b_t in (
                            ("q", qT[pr], bq_t[pr]),
                            ("k", kT[pr], bk_t[pr]),
                        ):
                            for sw in range(QW):
                                # bias-add alternates ACT/DVE: ACT is the
                                # busier engine once early exp groups start
                                if sw % 2 == 0:
                                    nc.scalar.activation(
                                        dst[:, sw * NQ : (sw + 1) * NQ],
                                        pq2[kind][sw][:],
                                        Act.Identity,
                                        bias=b_t[:],
                                    )
                                else:
                                    nc.vector.tensor_scalar_add(
                                        dst[:, sw * NQ : (sw + 1) * NQ],
                                        pq2[kind][sw][:],
                                        b_t[:],
                                    )

                # --- v passes: sw-major over both pairs, with the qw0/qw1p0
                # score groups interleaved as PE filler.  The sw0 step
                # finishes all heads' va[0..3] early, unblocking qw0's
                # o-matmuls right at attention start.  PSUM banks:
                # ps_v 2 + ps_t 2 + sg_early 4 = 8.
                ps_v = ph12.enter_context(
                    tc.tile_pool(name="ps_v", bufs=1, space="PSUM")
                )
                ps_t = ph12.enter_context(
                    tc.tile_pool(name="ps_t", bufs=2, space="PSUM")
                )
                sg_early = ph12.enter_context(
                    tc.tile_pool(name="sg_early", bufs=4, space="PSUM")
                )

                def early_group(qw, pr, kc):
                    j = kc - 4 * qw
                    qa = j * P if 0 < j < 4 else 0
                    e = epool.tile([P, 2 * NQ], bf16, tag="e", name="e")
                    for hh in range(2):
                        off = hh * D
                        sgh = sg_early.tile([P, NQ], f32, tag="sge", name="sgh")
                        nc.tensor.matmul(
                            sgh[:, qa:NQ],
                            kT[pr][off : off + D, kc * P : (kc + 1) * P],
                            qT[pr][off : off + D, qw * NQ + qa : (qw + 1) * NQ],
                            start=True,
                            stop=True,
                            skip_group_check=True,
                        )
                        nc.scalar.activation(
                            e[:, hh * NQ + qa : (hh + 1) * NQ],
                            sgh[:, qa:NQ],
                            Act.Exp,
                        )
                        if 0 <= j < 4:
                            nc.vector.tensor_mul(
                                e[:, hh * NQ + qa : (hh + 1) * NQ],
                                e[:, hh * NQ + qa : (hh + 1) * NQ],
                                mask_t[j][:, qa:NQ],
                            )
                    pre_es.setdefault((qw, pr), []).append(e)

                early_jobs = (
                    [(0, 0, kc) for kc in range(4)]
                    + [(0, 1, kc) for kc in range(4)]
                )
                ei = 0
                for sw in range(QW):
                    for pr in range(2):
                        pv = ps_v.tile([P, NQ], f32, tag=f"pv{pr}", name="pv")
                        for ec in range(EC):
                            nc.tensor.matmul(
                                pv[:],
                                wv_t[pr][:, ec * P : (ec + 1) * P],
                                xT[ec][:, sw * NQ : (sw + 1) * NQ],
                                start=(ec == 0),
                                stop=(ec == EC - 1),
                            )
                            if ec % 4 == 3 and ei < len(early_jobs):
                                early_group(*early_jobs[ei])
                                ei += 1
                        # v-path copies alternate ACT/DVE so neither engine's
                        # queue backs up into the attention phase ramp
                        vt = vtmp.tile([P, NQ], bf16, tag="vtmp", name="vt")
                        (nc.scalar.copy if pr == 0 else nc.vector.tensor_copy)(
                            vt[:], pv[:]
                        )
                        for i in range(NQ // P):
                            sc = sw * (NQ // P) + i
                            pvt = ps_t.tile([P, P], bf16, tag="ptr", name="pvt")
                            nc.tensor.transpose(
                                pvt[:], vt[:, i * P : (i + 1) * P], ident[:]
                            )
                            # one strided copy per transpose: both heads'
                            # 64-col blocks land around the ones columns
                            dst = va2[pr][sc].rearrange("p (h c) -> p h c", h=2)[:, :, 0:D]
                            (nc.scalar.copy if i % 2 == 0 else nc.vector.tensor_copy)(
                                dst,
                                pvt[:].rearrange("p (h c) -> p h c", h=2),
                            )
                            nc.vector.memset(
                                va2[pr][sc].rearrange("p (h c) -> p h c", h=2)[:, :, D : D + 1],
                                1.0,
                            )

            # ---------- Phases 3+4: attention + W_o ----------
            # Paired-head attention: both heads of a pair run their score
            # matmuls CONCURRENTLY as PE row-tiles (K=64 each, tile_position
            # (0,0)/(64,0) auto-derived from base partitions), into the two
            # halves of one [128, 2*NQ] PSUM group (2 adjacent banks), then a
            # single batched exp covers both heads.  o-matmuls lag 3 groups;
            # W_o chunks are drip-fed one per kc-iteration.  Epilogues use a
            # PE outer-product broadcast + reciprocal_approx_fast (no GpSimd,
            # no iterative reciprocal).  PSUM: sg 2x2 + po-ring 3 + wo 1 = 8.
            LAG = 4
            with contextlib.ExitStack() as ph34:
                ps_o = ph34.enter_context(
                    tc.tile_pool(name="ps_o", bufs=3, space="PSUM")
                )
                ps_wo = ph34.enter_context(
                    tc.tile_pool(name="ps_wo", bufs=1, space="PSUM")
                )
                rpool = ph34.enter_context(tc.tile_pool(name="rpool", bufs=2))
                obuf = ph34.enter_context(tc.tile_pool(name="obuf", bufs=6))
                ph_att = ph34.enter_context(contextlib.ExitStack())
                ps_sg = ph_att.enter_context(
                    tc.tile_pool(name="ps_sg", bufs=2, space="PSUM")
                )

                wo_queue = []  # pending W_o chunk closures (prev q-window)

                def mk_wo_chunks(qw, pr_order=(0, 1), pool=None, alt_copy=False):
                    chunks = []
                    for i in range(NQ // P):
                        sc = qw * (NQ // P) + i
                        for n in range(E // NQ):
                            ci = len(chunks)
                            def chunk(sc=sc, n=n, ci=ci, pr_order=pr_order, pool=pool or ps_wo):
                                pw = pool.tile([P, NQ], f32, tag="pwo", name="pw")
                                for step, pr in enumerate(pr_order):
                                    nc.tensor.matmul(
                                        pw[:],
                                        oT[pr][:, sc * P : (sc + 1) * P],
                                        wo_t[pr][:, n * NQ : (n + 1) * NQ],
                                        start=(step == 0),
                                        stop=(step == 1),
                                    )
                                ob = obuf.tile([P, NQ], bf16, tag="ob", name="ob")
                                # final flush splits copies ACT/DVE (ACT is
                                # idle at the tail) to drain the pipe faster
                                (nc.scalar.copy if alt_copy and ci % 2 else nc.vector.tensor_copy)(
                                    ob[:], pw[:]
                                )
                                nc.sync.dma_start(
                                    out_d[sc * P : (sc + 1) * P, n * NQ : (n + 1) * NQ],
                                    ob[:],
                                )
                            chunks.append(chunk)
                    return chunks

                ones64 = rpool.tile([1, D], f32r, tag="ones64", name="ones64")
                nc.vector.memset(ones64[:].bitcast(f32), 1.0)

                carry = None  # deferred tail (o-mms + epilogues) of prev pair
                for qw in range(QW):
                    nkc = 4 * qw + 4  # causal k-chunks for this q-window
                    # last window: end on pair 0 and accumulate the final W_o
                    # pair-1-first so its first matmuls don't wait on the
                    # very last epilogue
                    pair_order = (1, 0) if qw == QW - 1 else (0, 1)
                    for pr in pair_order:
                        po = [
                            ps_o.tile([D + 1, NQ], f32, tag="po", name=f"po{hh}")
                            for hh in range(2)
                        ]
                        es = [None] * nkc
                        sls = [None] * nkc

                        def emit_o(kc, po=po, pr=pr, es=es, sls=sls, nkc=nkc, heads=(0, 1)):
                            for hh in heads:
                                nc.tensor.matmul(
                                    po[hh][:, sls[kc]],
                                    va2[pr][kc][:, hh * (D + 1) : (hh + 1) * (D + 1)],
                                    es[kc][:, hh * NQ + sls[kc].start : hh * NQ + NQ],
                                    start=(kc == 0),
                                    stop=(kc == nkc - 1),
                                    skip_group_check=True,
                                )

                        def emit_epi(po=po, pr=pr, qw=qw, heads=(0, 1), fast=True):
                            # drow copy -> broadcast -> fast reciprocal ->
                            # normalize into oT.  Mid-kernel the broadcast
                            # rides the otherwise-idle GpSimd engine; the
                            # kernel-tail epilogue uses a PE outer product
                            # instead (GpSimd dispatch latency would sit
                            # exposed on the tail).
                            for hh in heads:
                                off = hh * D
                                drow = rpool.tile([1, NQ], f32r, tag="drow", name="drow")
                                nc.vector.tensor_copy(drow[:], po[hh][D : D + 1, :])
                                if fast:
                                    pb = ps_o.tile([D, NQ], f32, tag="po", name="pb")
                                    nc.tensor.matmul(
                                        pb[:], ones64[:], drow[:],
                                        start=True, stop=True,
                                    )
                                    rb = rpool.tile([D, NQ], f32, tag="rb", name="rb")
                                    nc.vector.reciprocal_approx_fast(rb[:], pb[:])
                                else:
                                    db = rpool.tile([D, NQ], f32r, tag="db", name="db")
                                    nc.gpsimd.partition_broadcast(db[:], drow[:])
                                    rb = rpool.tile([D, NQ], f32, tag="rb", name="rb")
                                    nc.vector.reciprocal_approx_fast(rb[:], db[:].bitcast(f32))
                                nc.vector.tensor_mul(
                                    oT[pr][off : off + D, qw * NQ : (qw + 1) * NQ],
                                    po[hh][0:D, :],
                                    rb[:],
                                )

                        pre = pre_es.get((qw, pr))
                        for idx, kc in enumerate(range(nkc)):
                            j = kc - 4 * qw
                            qa = j * P if 0 < j < 4 else 0
                            sl = slice(qa, NQ)
                            sls[kc] = sl
                            if pre is not None:
                                es[kc] = pre[kc]
                            else:
                                sg = ps_sg.tile([P, 2 * NQ], f32, tag="sg", name="sg")
                                for hh in range(2):
                                    off = hh * D
                                    nc.tensor.matmul(
                                        sg[:, hh * NQ + qa : (hh + 1) * NQ],
                                        kT[pr][off : off + D, kc * P : (kc + 1) * P],
                                        qT[pr][off : off + D, qw * NQ + qa : (qw + 1) * NQ],
                                        start=True,
                                        stop=True,
                                        skip_group_check=True,
                                    )
                                e = epool.tile([P, 2 * NQ], bf16, tag="e", name="e")
                                if qa > 0:
                                    # trimmed diagonal group: per-head exp (the
                                    # inter-head gap is unwritten PSUM)
                                    for hh in range(2):
                                        nc.scalar.activation(
                                            e[:, hh * NQ + qa : (hh + 1) * NQ],
                                            sg[:, hh * NQ + qa : (hh + 1) * NQ],
                                            Act.Exp,
                                        )
                                else:
                                    # one exp spanning both heads' PSUM banks
                                    nc.scalar.activation(e[:], sg[:], Act.Exp)
                                if 0 <= j < 4:
                                    for hh in range(2):
                                        nc.vector.tensor_mul(
                                            e[:, hh * NQ + qa : (hh + 1) * NQ],
                                            e[:, hh * NQ + qa : (hh + 1) * NQ],
                                            mask_t[j][:, sl],
                                        )
                                es[kc] = e
                            if idx == 0 and carry is not None:
                                carry()
                                carry = None
                            if idx >= 3 and wo_queue:
                                wo_queue.pop(0)()
                            if idx >= LAG:
                                emit_o(kc - LAG)

                        is_final = qw == QW - 1 and pr == pair_order[-1]

                        def mk_carry(emit_o=emit_o, emit_epi=emit_epi, nkc=nkc, final=is_final):
                            def c():
                                if final:
                                    # close each head separately so head 0's
                                    # epilogue overlaps head 1's tail o-mms
                                    for kc in range(nkc - LAG, nkc):
                                        emit_o(kc, heads=(0,))
                                    emit_epi(heads=(0,), fast=True)
                                    for kc in range(nkc - LAG, nkc):
                                        emit_o(kc, heads=(1,))
                                    emit_epi(heads=(1,), fast=True)
                                else:
                                    for kc in range(nkc - LAG, nkc):
                                        emit_o(kc)
                                    emit_epi()
                            return c

                        carry = mk_carry()
                    # window done: flush leftover W_o chunks of the previous
                    # window; the final window's chunks instead run on a
                    # wider pool after the score pool closes
                    for chunk in wo_queue:
                        chunk()
                    if qw < QW - 1:
                        wo_queue = mk_wo_chunks(qw, pr_order=(0, 1))
                if carry is not None:
                    carry()
                    carry = None
                # attention done: free the 4 score banks, run the final
                # window's W_o 4-deep pipelined so the tail stays dense
                ph_att.close()
                ps_wof = ph34.enter_context(
                    tc.tile_pool(name="ps_wof", bufs=4, space="PSUM")
                )
                for chunk in mk_wo_chunks(QW - 1, pr_order=(1, 0), pool=ps_wof, alt_copy=True):
                    chunk()

    nc.compile()
    return nc


def _host_shard(x, W_q, b_q, W_k, b_k, W_v, b_v, W_o, b_o):
    """Build the 8 per-core input maps. Returns (in_maps, b_o_eff)."""
    import ml_dtypes

    f32 = np.float32
    bf16 = ml_dtypes.bfloat16
    masks = np.zeros((4, P, NQ), dtype=f32)
    for j in range(4):
        for p in range(P):
            masks[j, p, j * P + p :] = 1.0
    # [P, 4*NQ] layout: 2KB-contiguous partition rows for the DMA
    masks = np.ascontiguousarray(masks.transpose(1, 0, 2).reshape(P, 4 * NQ))
    ident = np.eye(P, dtype=f32)

    def pack_w(wpair):
        # [E, 2D] -> [P, EC*P]: partition p holds chunk-row p of every
        # e-chunk, so each partition row is one contiguous DMA run
        return np.ascontiguousarray(
            wpair.reshape(EC, P, P).transpose(1, 0, 2).reshape(P, EC * P)
        )

    in_maps = []
    for c in range(N_CORES):
        b, g = c // 4, c % 4
        heads = [4 * g + i for i in range(4)]
        wq = np.zeros((2, P, EC * P), dtype=f32)
        wk = np.zeros((2, P, EC * P), dtype=f32)
        wv = np.zeros((2, P, EC * P), dtype=f32)
        bq = np.zeros((2, P, 1), dtype=f32)
        bk = np.zeros((2, P, 1), dtype=f32)
        wo = np.zeros((2, P, E), dtype=f32)
        for pr in range(2):
            h0, h1 = heads[2 * pr], heads[2 * pr + 1]
            wpair_q = np.concatenate([W_q[h0], W_q[h1]], axis=1) * 0.125
            wpair_k = np.concatenate([W_k[h0], W_k[h1]], axis=1)
            wpair_v = np.concatenate([W_v[h0], W_v[h1]], axis=1)
            wq[pr] = pack_w(wpair_q)
            wk[pr] = pack_w(wpair_k)
            wv[pr] = pack_w(wpair_v)
            bq[pr, :, 0] = np.concatenate([b_q[h0], b_q[h1]]) * 0.125
            bk[pr, :, 0] = np.concatenate([b_k[h0], b_k[h1]])
            wo[pr] = W_o[h0 * D : h0 * D + 2 * D]
        in_maps.append(
            {
                "xT": np.ascontiguousarray(x[b].T).astype(bf16),
                "wq": wq.astype(bf16),
                "wk": wk.astype(bf16),
                "wv": wv.astype(bf16),
                "bq": bq,
                "bk": bk,
                "wo": wo.astype(bf16),
                "masks": masks.astype(bf16),
                "ident": ident.astype(bf16),
            }
        )
    b_o_eff = (b_v.reshape(-1).astype(f32) @ W_o.astype(f32) + b_o).astype(f32)
    return in_maps, b_o_eff


_PROGRAM = None


def _run(in_maps, trace=False):
    from concourse.bass_utils import run_bass_kernel_spmd

    global _PROGRAM
    if _PROGRAM is None:
        _PROGRAM = _build_program()
    return run_bass_kernel_spmd(
        _PROGRAM, in_maps, core_ids=list(range(N_CORES)), trace=trace
    )


def kernel(x, W_q, b_q, W_k, b_k, W_v, b_v, W_o, b_o, _trace=False, _result_box=None):
    _ensure_axon_hooks()
    args = [np.asarray(a, dtype=np.float32) for a in (x, W_q, b_q, W_k, b_k, W_v, b_v, W_o, b_o)]
    in_maps, b_o_eff = _host_shard(*args)
    res = _run(in_maps, trace=_trace)
    if _result_box is not None:
        _result_box.append(res)
    B = x.shape[0]
    out = np.zeros((B, S, E), dtype=np.float32)
    for c in range(N_CORES):
        out[c // 4] += res.results[c]["out"].astype(np.float32)
    out += b_o_eff
    return out

